# revision 3
# baseline (speedup 1.0000x reference)
"""Trainium2 Bass kernel for a pre-LN transformer encoder block.

Model: y = x + FFN(LN2(x + Attn(LN1(x))))  with
  D_MODEL=1024, D_FF=4096, H=16 heads, B=4, S=2048, fp32 reference.

Wall time through the axon tunnel is dominated by host->device input bytes,
so the wire format is fp16 and nothing is replicated across cores:
  - weights are sent as 8 disjoint shards (3MB/core) and AllGathered on
    device into a shared DRAM blob (24MB, ~100us on NeuronLink);
  - x is sent as 8 disjoint 1024-token shards (2MB/core, feature-major);
    cores 2b and 2b+1 (which share batch element b) pair-AllGather their
    shards to reconstruct the batch's full 2048 tokens in natural order.

Compute per core c (b=c//2, r=c%2): LN1 + K/V over the gathered 2048
tokens, LN1 + Q over its own 1024 tokens (read straight from its input
shard, so no partition-dependent addressing is needed), all 16 heads of
attention for its 1024 queries, then wo / LN2 / FFN for those tokens.
All tensor-engine operands are fp16 (full PE rate), accumulation is fp32
PSUM, LN/softmax stats are fp32.  Biases are per-partition fp32; softmax
denominators come from an appended ones-column on V; the attention mask
folds into an extra contraction row of K (fp16-safe -50000 additive).
"""

import numpy as np

D = 1024          # d_model
H = 16            # heads
DKH = 64          # head dim
DFF = 4096
T = 2048          # tokens per batch element (keys)
TQ = 1024         # queries per core
MADD = -50000.0   # masked-key additive, fp16-safe; exp((q.k+MADD)/8) == 0
EPS = 1e-5
P = 128
WSH = 384         # weight-shard rows: 12M elements as [3072, 4096] / 8

_CACHE = {}


def _build_nc():
    import concourse.bass as bass
    import concourse.tile as tile
    import concourse.mybir as mybir
    from concourse import bacc
    from concourse.bass import ts

    fp32 = mybir.dt.float32
    fp16 = mybir.dt.float16
    AF = mybir.ActivationFunctionType
    OP = mybir.AluOpType

    nc = bacc.Bacc("TRN2", target_bir_lowering=False, debug=False, num_devices=8)

    # ---- kernel I/O (per core) ----
    xqh = nc.dram_tensor("xqh", [D, TQ], fp16, kind="ExternalInput").ap()
    wsh = nc.dram_tensor("wsh", [WSH, 4096], fp16, kind="ExternalInput").ap()
    mrow = nc.dram_tensor("mrow", [2, T], fp16, kind="ExternalInput").ap()
    ball = nc.dram_tensor("ball", [P, 64], fp32, kind="ExternalInput").ap()
    bvr = nc.dram_tensor("bvr", [1, D], fp32, kind="ExternalInput").ap()
    lnab = nc.dram_tensor("lnab", [1, 4], fp32, kind="ExternalInput").ap()
    yT = nc.dram_tensor("yT", [D, TQ], fp16, kind="ExternalOutput").ap()

    with tile.TileContext(nc) as tc:
        _emit(nc, tc, tile, mybir, ts, fp32, fp16, AF, OP, locals())
    nc.compile()
    return nc


def _emit(nc, tc, tile, mybir, ts, fp32, fp16, AF, OP, io):
    xqh, wsh, mrow = io["xqh"], io["wsh"], io["mrow"]
    ball, bvr, lnab, yT = io["ball"], io["bvr"], io["lnab"], io["yT"]

    from contextlib import ExitStack
    es = ExitStack()
    with es:
        es.enter_context(nc.allow_low_precision(
            reason="fp16 wire/compute is deliberate; fp32 psum accumulation"))
        dram = es.enter_context(tc.tile_pool(name="dram", bufs=1, space="DRAM"))
        consts = es.enter_context(tc.tile_pool(name="consts", bufs=1))
        stg = es.enter_context(tc.tile_pool(name="stg", bufs=6))
        rows = es.enter_context(tc.tile_pool(name="rows", bufs=8))

        # DRAM scratch + collective buffers
        xq_b = dram.tile([D, TQ], fp16, tag="xq_b")
        wsh_b = dram.tile([WSH, 4096], fp16, tag="wsh_b")
        xga = dram.tile([2 * D, TQ], fp16, tag="xga")
        wga = dram.tile([3072, 4096], fp16, tag="wga", addr_space="Shared")
        ktd = dram.tile([H, DKH + 1, T], fp16, tag="ktd")      # K^T + mask row
        qtd = dram.tile([H, DKH + 1, TQ], fp16, tag="qtd")     # Q^T + ones row
        ctxd = dram.tile([P, 8, TQ], fp16, tag="ctxd")         # ctx^T pair-chunked

        # stage shards into internal DRAM (collectives can't read kernel I/O)
        nc.sync.dma_start(xq_b[:], xqh[:])
        nc.sync.dma_start(wsh_b[:], wsh[:])
        nc.gpsimd.collective_compute(
            "AllGather", OP.bypass,
            replica_groups=[[0, 1], [2, 3], [4, 5], [6, 7]],
            ins=[xq_b.opt()], outs=[xga.opt()])
        nc.gpsimd.collective_compute(
            "AllGather", OP.bypass,
            replica_groups=[list(range(8))],
            ins=[wsh_b.opt()], outs=[wga.opt()])

        # views: weights from the gathered blob, x from the gathered pair
        def wmat(m):        # qkvo matrix m as [p, c, f] with row = c*128+p
            return wga[256 * m : 256 * (m + 1), :].rearrange(
                "(c pa) (pb f) -> (pa pb) c f", c=8, pa=32, pb=4, f=D)
        wq_v, wk_v, wv_v, wo_v = wmat(0), wmat(1), wmat(2), wmat(3)
        w1r = wga[1024:2048, :].rearrange("(c p) f -> p c f", p=P)
        w2r = wga[2048:3072, :].rearrange(
            "(j pa) (pb o) -> (pa pb) j o", j=32, pa=32, pb=4, o=D)
        xga_r = xga.rearrange("(h c p) t -> h p c t", h=2, c=8, p=P)
        xqr = xqh.rearrange("(c p) t -> p c t", p=P)
        yTr = yT.rearrange("(c p) t -> p c t", p=P)

        # ---- constants ----
        ball_sb = consts.tile([P, 64], fp32, tag="ball")
        nc.sync.dma_start(ball_sb[:], ball[:])
        bq_sb = ball_sb[:, 0:8]
        bk_sb = ball_sb[:, 8:16]
        bo_sb = ball_sb[:, 16:24]
        b2_sb = ball_sb[:, 24:32]
        b1_sb = ball_sb[:, 32:64]
        bv_sb = consts.tile([P, D], fp32, tag="bv")            # bv bcast on rows
        nc.sync.dma_start(bv_sb[:], bvr.to_broadcast((P, D)))
        lnab_sb = consts.tile([1, 4], fp32, tag="lnab")
        nc.sync.dma_start(lnab_sb[:], lnab[:])
        ln1_sb = lnab_sb[0:1, 0:2]
        ln2_sb = lnab_sb[0:1, 2:4]
        # memset writes fp32; stage then DVE-copy to fp16
        ones_f = consts.tile([P, P], fp32, tag="ones_f")
        nc.vector.memset(ones_f[:], 1.0)
        ones_c = consts.tile([P, 1], fp16, tag="ones_c")       # colsum lhsT
        nc.vector.tensor_copy(ones_c[:], ones_f[:, 0:1])
        ones_r = consts.tile([1, P], fp16, tag="ones_r")       # bcast lhsT
        nc.vector.tensor_copy(ones_r[:], ones_f[0:1, :])

        # mask row of K^T and ones row of Q^T
        for h in range(H):
            nc.sync.dma_start(ktd[h, DKH : DKH + 1, :], mrow[0:1, :])
            nc.sync.dma_start(qtd[h, DKH : DKH + 1, :], mrow[1:2, 0:TQ])

        def layer_norm_cols(x_src_fn, ab_sb, sB_ps, tB_ps, psp):
            """Emit LN stats for one 512-token chunk.

            x_src_fn(c) -> [128, 512] fp16 AP of input chunk c (c in 0..8).
            Fills sB_ps/tB_ps ([128,512] psum) with broadcast scale/shift:
            xn = x * sB - tB.
            """
            cx = psp.tile([1, 512], fp32, tag="sums", bufs=2)
            csq = psp.tile([1, 512], fp32, tag="sums", bufs=2)
            for c in range(8):
                nc.tensor.matmul(cx[:], ones_c[:], x_src_fn(c),
                                 start=(c == 0), stop=(c == 7))
            for c in range(8):
                sq = stg.tile([P, 512], fp16, tag="stg", name="sq")
                nc.vector.tensor_mul(sq[:], x_src_fn(c), x_src_fn(c))
                nc.tensor.matmul(csq[:], ones_c[:], sq[:],
                                 start=(c == 0), stop=(c == 7))
            mean = rows.tile([1, 512], fp32, tag="rows", name="mean")
            nc.vector.tensor_scalar_mul(mean[:], cx[:], 1.0 / D)
            m2s = rows.tile([1, 512], fp32, tag="rows", name="m2s")
            nc.vector.scalar_tensor_tensor(m2s[:], mean[:], float(D) / (D - 1),
                                           mean[:], op0=OP.mult, op1=OP.mult)
            var = rows.tile([1, 512], fp32, tag="rows", name="var")
            nc.vector.scalar_tensor_tensor(var[:], csq[:], 1.0 / (D - 1),
                                           m2s[:], op0=OP.mult, op1=OP.subtract)
            std = rows.tile([1, 512], fp32, tag="rows", name="std")
            nc.scalar.activation(std[:], var[:], AF.Sqrt)
            nc.vector.tensor_scalar_add(std[:], std[:], EPS)
            rstd = rows.tile([1, 512], fp32, tag="rows", name="rstd")
            nc.vector.reciprocal(rstd[:], std[:])
            s_r = rows.tile([1, 512], fp16, tag="rows", name="s_r")
            nc.vector.tensor_scalar_mul(s_r[:], rstd[:], ab_sb[0:1, 0:1])
            t_r = rows.tile([1, 512], fp16, tag="rows", name="t_r")
            nc.vector.tensor_mul(t_r[:], mean[:], s_r[:])
            nc.vector.tensor_scalar_sub(t_r[:], t_r[:], ab_sb[0:1, 1:2])
            nc.tensor.matmul(sB_ps[:], ones_r[:], s_r[:], start=True, stop=True)
            nc.tensor.matmul(tB_ps[:], ones_r[:], t_r[:], start=True, stop=True)

        # ================= P0: LN1 + K/V (full batch) + Q (my tokens) ======
        with tc.tile_pool(name="p0big", bufs=1) as p0big:
            v_sb = p0big.tile([P, 16, H * (DKH + 1)], fp16, tag="vaug")
            # ones columns of V_aug (col 64 of each head block)
            vv = v_sb.rearrange("p t (h e) -> p t h e", e=DKH + 1)
            nc.vector.tensor_copy(
                vv[:, :, :, DKH : DKH + 1],
                ones_f[:, 0:1].to_broadcast((P, 16, H, 1)))

            with tc.tile_pool(name="p0", bufs=2) as p0, \
                 tc.tile_pool(name="ps0", bufs=1, space="PSUM") as ps0:
                for tci in range(4):          # KV pass over gathered batch
                    tsl = ts(tci, 512)
                    x_sb = p0.tile([P, 8, 512], fp16, tag="xchunk")
                    nc.sync.dma_start(x_sb[:],
                                      xga_r[tci // 2, :, :, ts(tci % 2, 512)])
                    sB = ps0.tile([P, 512], fp32, tag="bcast", bufs=2)
                    tB = ps0.tile([P, 512], fp32, tag="bcast", bufs=2)
                    layer_norm_cols(lambda c: x_sb[:, c, :], ln1_sb, sB, tB, ps0)
                    xn_sb = p0.tile([P, 8, 512], fp16, tag="xnchunk")
                    for c in range(8):
                        nc.vector.tensor_mul(xn_sb[:, c, :], x_sb[:, c, :], sB[:])
                        nc.vector.tensor_sub(xn_sb[:, c, :], xn_sb[:, c, :], tB[:])

                    # K projection (transposed out)
                    for dkb in range(2):
                        wb = p0.tile([P, 8, 512], fp16, tag="wblk")
                        nc.sync.dma_start(wb[:], wk_v[:, :, ts(dkb, 512)])
                        for dkc in range(4):
                            g = dkb * 4 + dkc
                            kps = ps0.tile([P, 512], fp32, tag="mm", bufs=4)
                            for c in range(8):
                                nc.tensor.matmul(kps[:], wb[:, c, ts(dkc, P)],
                                                 xn_sb[:, c, :],
                                                 start=(c == 0), stop=(c == 7))
                            kst = stg.tile([P, 512], fp16, tag="stg", name="kst")
                            nc.vector.tensor_scalar_add(kst[:], kps[:],
                                                        bk_sb[:, g : g + 1])
                            nc.sync.dma_start(ktd[2 * g, 0:DKH, tsl],
                                              kst[0:DKH, :])
                            nc.sync.dma_start(ktd[2 * g + 1, 0:DKH, tsl],
                                              kst[DKH:P, :])

                    # V projection (natural out), augmented layout
                    for dvb in range(2):
                        wb = p0.tile([P, 8, 512], fp16, tag="wblk")
                        nc.sync.dma_start(wb[:], wv_v[:, :, ts(dvb, 512)])
                        for tsub in range(4):
                            tcc = tci * 4 + tsub
                            vps = ps0.tile([P, 512], fp32, tag="mm", bufs=4)
                            for c in range(8):
                                nc.tensor.matmul(vps[:], xn_sb[:, c, ts(tsub, P)],
                                                 wb[:, c, :],
                                                 start=(c == 0), stop=(c == 7))
                            vdst = v_sb[:, tcc, dvb * 8 * (DKH + 1) :
                                        (dvb + 1) * 8 * (DKH + 1)]
                            vdst = vdst.rearrange("p (h e) -> p h e", e=DKH + 1)
                            bsl = bv_sb[:, ts(dvb, 512)].rearrange(
                                "p (h e) -> p h e", e=DKH)
                            nc.vector.tensor_add(
                                vdst[:, :, 0:DKH],
                                vps.rearrange("p (h e) -> p h e", e=DKH),
                                bsl)

                for tqi in range(2):          # Q pass over my own tokens
                    tsl = ts(tqi, 512)
                    xq_sb = p0.tile([P, 8, 512], fp16, tag="xchunk")
                    nc.sync.dma_start(xq_sb[:], xqr[:, :, tsl])
                    sB = ps0.tile([P, 512], fp32, tag="bcast", bufs=2)
                    tB = ps0.tile([P, 512], fp32, tag="bcast", bufs=2)
                    layer_norm_cols(lambda c: xq_sb[:, c, :], ln1_sb, sB, tB, ps0)
                    xnq_sb = p0.tile([P, 8, 512], fp16, tag="xnchunk")
                    for c in range(8):
                        nc.vector.tensor_mul(xnq_sb[:, c, :], xq_sb[:, c, :], sB[:])
                        nc.vector.tensor_sub(xnq_sb[:, c, :], xnq_sb[:, c, :], tB[:])
                    for dkb in range(2):
                        wb = p0.tile([P, 8, 512], fp16, tag="wblk")
                        nc.sync.dma_start(wb[:], wq_v[:, :, ts(dkb, 512)])
                        for dkc in range(4):
                            g = dkb * 4 + dkc
                            kps = ps0.tile([P, 512], fp32, tag="mm", bufs=4)
                            for c in range(8):
                                nc.tensor.matmul(kps[:], wb[:, c, ts(dkc, P)],
                                                 xnq_sb[:, c, :],
                                                 start=(c == 0), stop=(c == 7))
                            kst = stg.tile([P, 512], fp16, tag="stg", name="kst")
                            nc.vector.tensor_scalar_add(kst[:], kps[:],
                                                        bq_sb[:, g : g + 1])
                            nc.sync.dma_start(qtd[2 * g, 0:DKH, tsl],
                                              kst[0:DKH, :])
                            nc.sync.dma_start(qtd[2 * g + 1, 0:DKH, tsl],
                                              kst[DKH:P, :])

            # ================= P1: attention =================
            with tc.tile_pool(name="p1", bufs=2) as p1, \
                 tc.tile_pool(name="pr", bufs=4) as prp, \
                 tc.tile_pool(name="ps1", bufs=1, space="PSUM") as ps1:
                for h in range(H):
                    kt_sb = p1.tile([DKH + 1, T], fp16, tag="kt")
                    nc.sync.dma_start(kt_sb[:], ktd[h])
                    qh_sb = p1.tile([DKH + 1, TQ], fp16, tag="qh")
                    nc.sync.dma_start(qh_sb[:], qtd[h])
                    for qt in range(2):
                        qsl = ts(qt, 512)
                        ctx = ps1.tile([DKH + 1, 512], fp32, tag="ctx", bufs=2)
                        for kc2 in range(8):
                            sc = ps1.tile([P, 2, 512], fp32, tag="sc", bufs=2)
                            for j in range(2):
                                kc = 2 * kc2 + j
                                nc.tensor.matmul(sc[:, j, :],
                                                 kt_sb[:, ts(kc, P)],
                                                 qh_sb[:, qsl],
                                                 start=True, stop=True)
                            pr = prp.tile([P, 2, 512], fp16, tag="pr")
                            nc.scalar.activation(pr[:], sc[:], AF.Exp,
                                                 scale=1.0 / 8.0)
                            for j in range(2):
                                kc = 2 * kc2 + j
                                nc.tensor.matmul(
                                    ctx[:],
                                    v_sb[:, kc, h * (DKH + 1) : (h + 1) * (DKH + 1)],
                                    pr[:, j, :],
                                    start=(kc == 0), stop=(kc == 15))
                        # normalize by the denominator row and store ctx^T
                        rr = rows.tile([1, 512], fp16, tag="rows", name="rr")
                        nc.vector.reciprocal(rr[:], ctx[DKH : DKH + 1, :])
                        rb = ps1.tile([DKH, 512], fp32, tag="rb", bufs=2)
                        nc.tensor.matmul(rb[:], ones_r[0:1, 0:DKH], rr[:],
                                         start=True, stop=True)
                        cst = stg.tile([P, 512], fp16, tag="stg", name="cst")
                        nc.vector.tensor_copy(cst[0:DKH, :], ctx[0:DKH, :])
                        nc.vector.tensor_mul(cst[0:DKH, :], cst[0:DKH, :], rb[:])
                        nc.sync.dma_start(
                            ctxd[DKH * (h % 2) : DKH * (h % 2) + DKH, h // 2, qsl],
                            cst[0:DKH, :])

        # ================= P2: wo projection + residual =================
        with tc.tile_pool(name="p23", bufs=1) as p23:
            outT = p23.tile([P, 8, TQ], fp16, tag="outT")
            with tc.tile_pool(name="p2", bufs=1) as p2, \
                 tc.tile_pool(name="p2s", bufs=2) as p2s, \
                 tc.tile_pool(name="ps2", bufs=1, space="PSUM") as ps2:
                wo_sb = p2.tile([P, 8, D], fp16, tag="wo")
                nc.sync.dma_start(wo_sb[:], wo_v)
                for qt in range(2):
                    qsl = ts(qt, 512)
                    ccs = []
                    for c in range(8):
                        cc = p2s.tile([P, 512], fp16, tag="ctxc", bufs=10,
                                      name="cc")
                        nc.sync.dma_start(cc[:], ctxd[:, c, qsl])
                        ccs.append(cc)
                    for do in range(8):
                        ops_ = ps2.tile([P, 512], fp32, tag="mm", bufs=4)
                        for c in range(8):
                            nc.tensor.matmul(ops_[:], wo_sb[:, c, ts(do, P)],
                                             ccs[c][:],
                                             start=(c == 0), stop=(c == 7))
                        xq = p2s.tile([P, 512], fp16, tag="xq", bufs=2, name="xq")
                        nc.sync.dma_start(xq[:], xqr[:, do, qsl])
                        nc.vector.scalar_tensor_tensor(
                            outT[:, do, qsl], ops_[:], bo_sb[:, do : do + 1],
                            xq[:], op0=OP.add, op1=OP.add)

            # ================= P3: LN2 =================
            with tc.tile_pool(name="p3", bufs=1) as p3:
                xn2 = p3.tile([P, 8, TQ], fp16, tag="xn2")
                with tc.tile_pool(name="ps3", bufs=1, space="PSUM") as ps3:
                    for tci in range(2):
                        tsl = ts(tci, 512)
                        sB = ps3.tile([P, 512], fp32, tag="bcast", bufs=2)
                        tB = ps3.tile([P, 512], fp32, tag="bcast", bufs=2)
                        layer_norm_cols(lambda c: outT[:, c, tsl], ln2_sb,
                                        sB, tB, ps3)
                        for c in range(8):
                            nc.vector.tensor_mul(xn2[:, c, tsl],
                                                 outT[:, c, tsl], sB[:])
                            nc.vector.tensor_sub(xn2[:, c, tsl],
                                                 xn2[:, c, tsl], tB[:])

                # ================= P4: FFN + residual =================
                with tc.tile_pool(name="p4", bufs=1) as p4, \
                     tc.tile_pool(name="p4w", bufs=3) as p4w, \
                     tc.tile_pool(name="ps4", bufs=1, space="PSUM") as ps4:
                    h1_sb = p4.tile([P, 16, TQ], fp16, tag="h1")
                    for half in range(2):
                        # h1 = relu(w1^T xn2 + b1) for this dff half
                        for fb in range(8):           # 256-wide dff blocks
                            fof = half * 2048 + fb * 256
                            w1b = p4w.tile([P, 8, 256], fp16, tag="wstr",
                                           name="w1b")
                            nc.sync.dma_start(w1b[:],
                                              w1r[:, :, fof : fof + 256])
                            for fc in range(2):
                                f = fb * 2 + fc      # 0..15 within half
                                for qt in range(2):
                                    qsl = ts(qt, 512)
                                    hps = ps4.tile([P, 512], fp32, tag="h1m",
                                                   bufs=4)
                                    for c in range(8):
                                        nc.tensor.matmul(
                                            hps[:], w1b[:, c, ts(fc, P)],
                                            xn2[:, c, qsl],
                                            start=(c == 0), stop=(c == 7))
                                    nc.vector.tensor_scalar(
                                        h1_sb[:, f, qsl], hps[:],
                                        b1_sb[:, half * 16 + f : half * 16 + f + 1],
                                        0.0, op0=OP.add, op1=OP.max)
                        # h2 partial = w2^T h1 (+ b2 + residual on half 0)
                        for do in range(8):
                            w2c = p4w.tile([P, 16, P], fp16, tag="wstr",
                                           name="w2c")
                            nc.sync.dma_start(
                                w2c[:],
                                w2r[:, half * 16 : half * 16 + 16, ts(do, P)])
                            for qt in range(2):
                                qsl = ts(qt, 512)
                                h2p = ps4.tile([P, 512], fp32, tag="h2m", bufs=4)
                                for j in range(16):
                                    nc.tensor.matmul(h2p[:], w2c[:, j, :],
                                                     h1_sb[:, j, qsl],
                                                     start=(j == 0),
                                                     stop=(j == 15))
                                if half == 0:
                                    nc.vector.scalar_tensor_tensor(
                                        outT[:, do, qsl], h2p[:],
                                        b2_sb[:, do : do + 1],
                                        outT[:, do, qsl],
                                        op0=OP.add, op1=OP.add)
                                else:
                                    yst = stg.tile([P, 512], fp16, tag="stg",
                                                   name="yst")
                                    nc.vector.tensor_add(yst[:], h2p[:],
                                                         outT[:, do, qsl])
                                    nc.sync.dma_start(yTr[:, do, qsl], yst[:])


def _get_nc():
    if "nc" not in _CACHE:
        _CACHE["nc"] = _build_nc()
    return _CACHE["nc"]


def _make_in_maps(x, src_mask, wq, bq, wk, bk, wv, bv, wo, bo,
                  w1, b1, w2, b2, ln1_a, ln1_b, ln2_a, ln2_b):
    f = np.float32
    h = np.float16

    def chunk_bias(b, nchunk):
        return np.asarray(b, f).reshape(nchunk, P).T

    ball = np.empty((P, 64), f)
    ball[:, 0:8] = chunk_bias(bq, 8)
    ball[:, 8:16] = chunk_bias(bk, 8)
    ball[:, 16:24] = chunk_bias(bo, 8)
    ball[:, 24:32] = chunk_bias(b2, 8)
    ball[:, 32:64] = chunk_bias(b1, 32)
    ball = np.ascontiguousarray(ball)

    wflat = np.concatenate([
        np.asarray(wq, f).reshape(-1), np.asarray(wk, f).reshape(-1),
        np.asarray(wv, f).reshape(-1), np.asarray(wo, f).reshape(-1),
        np.asarray(w1, f).reshape(-1), np.asarray(w2, f).reshape(-1),
    ]).astype(h).reshape(3072, 4096)

    common = {
        "bvr": np.asarray(bv, f).reshape(1, D),
        "ball": ball,
        "lnab": np.array([[ln1_a.reshape(-1)[0], ln1_b.reshape(-1)[0],
                           ln2_a.reshape(-1)[0], ln2_b.reshape(-1)[0]]], f),
    }
    mrows = []
    for b in range(4):
        madd = np.where(np.asarray(src_mask[b]).reshape(T) == 0,
                        h(MADD), h(0.0)).astype(h)
        mrows.append(np.stack([madd, np.ones(T, h)]))
    in_maps = []
    for c in range(8):
        b, r = c // 2, c % 2
        m = dict(common)
        m["xqh"] = np.ascontiguousarray(
            np.asarray(x[b, r * TQ : (r + 1) * TQ, :], f).T.astype(h))
        m["wsh"] = np.ascontiguousarray(wflat[WSH * c : WSH * (c + 1)])
        m["mrow"] = mrows[b]
        in_maps.append(m)
    return in_maps


def kernel(**inputs):
    from concourse import bass_utils

    nc = _get_nc()
    in_maps = _make_in_maps(**inputs)
    res = bass_utils.run_bass_kernel_spmd(nc, in_maps, core_ids=list(range(8)))
    B = 4
    out = np.empty((B, T, D), np.float32)
    for c in range(8):
        b, r = c // 2, c % 2
        out[b, r * TQ : (r + 1) * TQ, :] = res.results[c]["yT"].T
    return out
